# revision 22
# baseline (speedup 1.0000x reference)
"""Multi-head attention (B=4, N=2048, C=1024, H=16, Dh=64) on 8 TRN2 NeuronCores.

Sharding: tensor-parallel over heads — core c owns heads (2c, 2c+1) for all
batches.  Each core computes its 2 heads' QKV projection, attention, and the
partial output projection (contraction over its 128 head-dims of w_proj);
the host sums the 8 partial projections and adds the bias.

Per-core pipeline (unit = one batch of 2048 tokens):
  - host passes xT = x^T [1024, 8192] so channels land on SBUF partitions
  - QT/KT/VT computed as [128(d, 2 heads stacked), t] tiles
  - scores computed TRANSPOSED: ST[k, q] = KT_h.T @ QT_h (contraction d=64,
    two heads row-packed into the PE array: h0 rows 0-63, h1 rows 64-127,
    running concurrently via tile_position row groups)
  - softmax without max-subtraction (scores verified: |s|*scale < 10):
    ACT exp reads the score PSUM pair [128, 1024] directly, writes PT
  - AV: O^T[d, q] with lhsT = [V_h | ones] (M=65): PSUM row 64 accumulates
    the softmax denominator for free; the accumulator is evicted to SBUF
    immediately (frees the PSUM bank) and normalized off the critical path
  - proj: out[t, o] = OT_tile.T @ wpT, evicted right after each q-span

Scheduling: the kernel is a software pipeline clocked by the Scalar
engine's exp (~1.06us per k-chunk).  Attention is emitted in 2-k-chunk
super-slots: both score pairs back-to-back (the second pair's LDWEIGHTS
hides under the first — K=64 row-group reuse), then a cost-budgeted
amount of filler (next unit's QKV / V-transposes / previous q-span's
proj, at SINGLE-matmul granularity), then the AV pairs of the PREVIOUS
super-slot (staggered so the in-order PE queue never waits on ACT).

MHA_DTYPE env: "bf16" (default) or "f32r" or "f32" — matmul input dtype.
PSUM accumulation and softmax statistics are always fp32.
"""

import os
import numpy as np

B, N, C = 4, 2048, 1024
H, Dh = 16, 64
NT = B * N            # 8192 tokens
NCORES = 8
HPC = H // NCORES     # 2 heads per core
SCALE = Dh ** -0.5

TPU = N               # tokens per unit (one batch)
QS = 512              # q-span
KC = 128              # k-chunk
MHA_DTYPE = os.environ.get("MHA_DTYPE", "bf16")
FILL_BUDGET = float(os.environ.get("MHA_FILL", "4.4"))
FILL0_BUDGET = float(os.environ.get("MHA_FILL0", "10.0"))  # unit-0 span
PAIRK = int(os.environ.get("MHA_PAIRK", "2"))      # k-chunks per super-slot
WSPLIT = os.environ.get("MHA_WSPLIT", "1") == "1"  # per-cc weight tiles
WARMUP_MMS = int(os.environ.get("MHA_WARMUP", "16"))
POP_START = int(os.environ.get("MHA_POP_START", "8"))
POP_EVERY = int(os.environ.get("MHA_POP_EVERY", "5"))

_CACHE = {}


def _np_in_dtype():
    if MHA_DTYPE == "bf16":
        import ml_dtypes
        return np.dtype(ml_dtypes.bfloat16)
    return np.dtype(np.float32)


def _build_program():
    import concourse.bacc as bacc
    import concourse.bass as bass
    import concourse.tile as tile
    from concourse import mybir
    from concourse.masks import make_identity

    f32 = mybir.dt.float32
    din = {
        "bf16": mybir.dt.bfloat16,
        "f32r": mybir.dt.float32r,
        "f32": mybir.dt.float32,
    }[MHA_DTYPE]

    nc = bacc.Bacc("TRN2", target_bir_lowering=False, debug=False)

    xT = nc.dram_tensor("xT", [C, NT], din, kind="ExternalInput").ap()
    wqkvT = nc.dram_tensor("wqkvT", [C, 6 * Dh], din, kind="ExternalInput").ap()
    wpT = nc.dram_tensor("wpT", [2 * Dh, C], din, kind="ExternalInput").ap()
    out = nc.dram_tensor("out", [NT, C], f32, kind="ExternalOutput").ap()

    NCC = C // 128        # 8 c-chunks
    NTT = TPU // QS       # 4 t-tiles per unit
    NKC = TPU // KC       # 16 k-chunks per unit
    NQS = TPU // QS       # 4 q-spans per unit
    KPT = QS // KC        # 4 k-chunks per t-tile
    VW = 2 * (Dh + 1)     # 130: V_sb row layout [V_h0 | 1 | V_h1 | 1]

    with tile.TileContext(nc) as tc:
        with (
            tc.tile_pool(name="const", bufs=1) as const,
            tc.tile_pool(name="xp", bufs=48) as xp,
            tc.tile_pool(name="qt", bufs=2) as qtp,
            tc.tile_pool(name="kt", bufs=2) as ktp,
            tc.tile_pool(name="vt", bufs=2) as vtp,
            tc.tile_pool(name="vsb", bufs=2) as vsbp,
            tc.tile_pool(name="pt", bufs=4) as ptp,
            tc.tile_pool(name="ot", bufs=2) as otp,
            tc.tile_pool(name="rn", bufs=2) as rnp,
            tc.tile_pool(name="po", bufs=4) as pop,
            tc.tile_pool(name="mps", bufs=2, space="PSUM") as mps,
            tc.tile_pool(name="stps", bufs=2, space="PSUM") as stps,
            tc.tile_pool(name="avps", bufs=1, space="PSUM") as avps,
        ):
            ident = const.tile([128, 128], din)
            make_identity(nc, ident)

            # ramp: pull the ACT exp-table load (~2.7us) off the critical
            # path, and keep the PE busy during the initial x/w DMAs so the
            # HAM clock-gate is released before the first real matmul
            def warm_mm():
                # real matmul (transpose-mode does NOT engage the HAM
                # clock-gate): keeps/brings the PE at 2.4 GHz
                wps = mps.tile([128, 128], f32, tag="m", name="wps")
                nc.tensor.matmul(wps, ident, ident, skip_group_check=True)

            if WARMUP_MMS > 0:
                warm_out = const.tile([128, 8], f32)
                nc.scalar.activation(
                    warm_out, ident[:, 0:8],
                    mybir.ActivationFunctionType.Exp)
                for _ in range(WARMUP_MMS):
                    warm_mm()

            # per-cc weight tiles: first QKV matmul only waits on its own chunk
            wq_sbs = []
            if WSPLIT:
                for cc in range(NCC):
                    w_t = const.tile([128, 6 * Dh], din, tag=f"w_t{cc}",
                                     name=f"w_t{cc}")
                    nc.gpsimd.dma_start(
                        out=w_t,
                        in_=wqkvT[cc * 128:(cc + 1) * 128, :],
                    )
                    wq_sbs.append(w_t)
            else:
                wq_sb = const.tile([128, NCC * 6 * Dh], din)
                for cc in range(NCC):
                    nc.gpsimd.dma_start(
                        out=wq_sb[:, cc * 6 * Dh:(cc + 1) * 6 * Dh],
                        in_=wqkvT[cc * 128:(cc + 1) * 128, :],
                    )
                    wq_sbs.append(wq_sb[:, cc * 6 * Dh:(cc + 1) * 6 * Dh])
            # wp DMA is issued AFTER unit-0 t-tile-0's x DMAs (below):
            # same gpsimd queue, and wp isn't needed until the first proj
            wp_sb = const.tile([128, C], din)
            wp_loaded = [False]

            def load_wp():
                if not wp_loaded[0]:
                    wp_loaded[0] = True
                    nc.gpsimd.dma_start(out=wp_sb, in_=wpT)

            # per-unit persistent tiles, allocated lazily
            QT, KT, VT, VSB, OT = {}, {}, {}, {}, {}

            def alloc_unit(u):
                QT[u] = qtp.tile([128, TPU], din, tag="QT", name=f"QT{u}")
                KT[u] = ktp.tile([128, TPU], din, tag="KT", name=f"KT{u}")
                VT[u] = vtp.tile([128, TPU], din, tag="VT", name=f"VT{u}")
                VSB[u] = vsbp.tile([128, NKC * VW], din, tag="VSB", name=f"VSB{u}")

            def qkv_items(u, tt, early_load=False, split_q=False):
                """QKV projection + V transposes for t-tile tt of unit u, as
                (cost, closure) items at single-matmul granularity.  With
                early_load the x DMAs are issued immediately (slots before
                the matmuls drain) so their latency is hidden instead of
                stalling the first matmul of the group.

                Groups are emitted K, V, transposes, Q: attention on q-span
                qs only reads the Q of t-tile qs, but the K/V of ALL
                t-tiles, so K/V are the critical-path part.  With split_q
                the Q matmuls are returned separately (lower priority:
                unit-0 t-tile tt's Q is not needed until span (0, tt))."""
                items = []
                q_items = []
                state = {}

                def load_x():
                    if tt == 0:
                        alloc_unit(u)
                    t0 = u * TPU
                    cells = []
                    for cc in range(NCC):
                        xt = xp.tile([128, QS], din, tag="xs", name="xt")
                        # gpsimd DMA queue: the sync queue carries the
                        # output DMAs and backs up ~20-30us, which would
                        # defeat the early load
                        nc.gpsimd.dma_start(
                            out=xt,
                            in_=xT[cc * 128:(cc + 1) * 128,
                                   t0 + tt * QS:t0 + (tt + 1) * QS],
                        )
                        cells.append(xt)
                    state["xs"] = cells

                if early_load:
                    load_x()
                else:
                    items.append((0.0, load_x))

                def qkv_mm(grp, cc):
                    def run():
                        if cc == 0:
                            state[grp] = mps.tile([128, QS], f32, tag="m",
                                                  name="ps")
                        ps = state[grp]
                        w_sl = wq_sbs[cc][:, grp * 128:(grp + 1) * 128]
                        nc.tensor.matmul(
                            ps, w_sl, state["xs"][cc],
                            start=(cc == 0), stop=(cc == NCC - 1),
                            skip_group_check=True,
                        )
                        if cc == NCC - 1:
                            tgt = (QT, KT, VT)[grp][u]
                            nc.vector.tensor_copy(
                                tgt[:, tt * QS:(tt + 1) * QS], ps)
                    return run

                for cc in range(NCC):
                    items.append((1.0, qkv_mm(1, cc)))     # K
                if u == 0 and tt == 0:
                    # startup critical path: the first scores need K+Q
                    for cc in range(NCC):
                        items.append((1.0, qkv_mm(0, cc)))  # Q
                for cc in range(NCC):
                    items.append((1.0, qkv_mm(2, cc)))     # V
                if not (u == 0 and tt == 0):
                    for cc in range(NCC):
                        (q_items if split_q else items).append(
                            (1.0, qkv_mm(0, cc)))          # Q

                def transpose_item(j):
                    def run():
                        kc = tt * KPT + j
                        tp = mps.tile([128, 128], din, tag="m", name="tp")
                        nc.tensor.transpose(
                            tp, VT[u][:, kc * 128:(kc + 1) * 128], ident)
                        base = kc * VW
                        nc.vector.tensor_copy(
                            VSB[u][:, base: base + Dh], tp[:, 0:Dh])
                        nc.vector.memset(
                            VSB[u][:, base + Dh: base + Dh + 1], 1.0)
                        nc.vector.tensor_copy(
                            VSB[u][:, base + Dh + 1: base + 2 * Dh + 1],
                            tp[:, Dh: 2 * Dh])
                        nc.vector.memset(
                            VSB[u][:, base + 2 * Dh + 1: base + VW], 1.0)
                    return run

                # transposes right after V (before Q): AVs need VSB one
                # slot after their scores
                if u == 0 and tt == 0:
                    for j in range(KPT):
                        items.append((0.8, transpose_item(j)))
                else:
                    base = (0 if early_load else 1) + 2 * NCC
                    for j in range(KPT):
                        items.insert(base + j, (0.8, transpose_item(j)))
                if split_q:
                    return items, q_items
                return items

            hard_items = []   # qkv work: must drain before its unit's attn
            med_items = []    # unit-0 deferred Q matmuls
            soft_items = []   # normalize/proj: anytime
            carry = [0.0]     # fractional pump budget carry

            def pump_budget(budget):
                """Pop items worth ~budget matmul-equivalents of PE time."""
                c = carry[0] + budget
                while c > 0 and (hard_items or med_items or soft_items):
                    q = (hard_items if hard_items
                         else med_items if med_items else soft_items)
                    cost, fn = q.pop(0)
                    fn()
                    c -= cost if cost > 0 else 0.0
                    if cost == 0.0:
                        continue
                carry[0] = (min(c, 0.0)
                            if (hard_items or med_items or soft_items)
                            else 0.0)

            def pump_all_hard_only():
                while hard_items:
                    _, fn = hard_items.pop(0)
                    fn()

            def pump_all_hard():
                pump_all_hard_only()
                while med_items:
                    _, fn = med_items.pop(0)
                    fn()

            def pump_med_n(n):
                while med_items and n > 0:
                    _, fn = med_items.pop(0)
                    fn()
                    n -= 1

            def pump_all():
                pump_all_hard()
                while soft_items:
                    _, fn = soft_items.pop(0)
                    fn()

            # ---- attention: 2-k-chunk super-slots, AVs staggered one
            # super-slot behind their exp so PE never stalls on ACT; score
            # pairs of adjacent k-chunks run back-to-back so the second
            # pair's LDWEIGHTS hides under the first (K=64 row-group reuse)
            av_pending = [[]]

            def emit_scores_exp(u, qs, kc):
                q0 = qs * QS
                sp = stps.tile([128, 2 * QS], f32, name="sp")
                nc.tensor.matmul(
                    sp[:, 0:QS],
                    KT[u][0:Dh, kc * 128:(kc + 1) * 128],
                    QT[u][0:Dh, q0:q0 + QS],
                    skip_group_check=True,
                )
                nc.tensor.matmul(
                    sp[:, QS:2 * QS],
                    KT[u][Dh:128, kc * 128:(kc + 1) * 128],
                    QT[u][Dh:128, q0:q0 + QS],
                    skip_group_check=True,
                )
                pt = ptp.tile([128, 2 * QS], din, name="pt")
                nc.scalar.activation(
                    pt, sp, mybir.ActivationFunctionType.Exp,
                    scale=SCALE,
                )
                return pt

            def make_av(u, oh, kc, pt):
                def run():
                    for i in range(2):
                        vbase = kc * VW + i * (Dh + 1)
                        nc.tensor.matmul(
                            oh[i],
                            VSB[u][:, vbase: vbase + Dh + 1],
                            pt[:, i * QS:(i + 1) * QS],
                            start=(kc == 0), stop=(kc == NKC - 1),
                            skip_group_check=True,
                        )
                return run

            def evict_oh(oh):
                """Evict AV accumulators to SBUF (frees PSUM)."""
                osbs = []
                for i in range(2):
                    osb = rnp.tile([Dh + 1, QS], f32, tag=f"osb{i}",
                                   name=f"osb{i}")
                    nc.vector.tensor_copy(osb, oh[i])
                    osbs.append(osb)
                return osbs

            def normalize_items(u, qs, osbs, c0=0, c1=QS, warm=False):
                if qs == 0 and c0 == 0:
                    OT[u] = otp.tile([128, TPU], din, tag="OT",
                                     name=f"OT{u}")
                q0 = qs * QS
                W = c1 - c0

                def norm(i):
                    def run():
                        osb = osbs[i]
                        d_row = rnp.tile([1, W], f32, tag=f"d{W}",
                                         name="d_row")
                        nc.vector.tensor_copy(d_row, osb[Dh:Dh + 1, c0:c1])
                        r_row = rnp.tile([1, W], f32, tag=f"r{W}",
                                         name="r_row")
                        nc.vector.reciprocal_approx_fast(r_row, d_row)
                        Rb = rnp.tile([Dh, W], f32, tag=f"R{W}", name="Rb")
                        nc.gpsimd.partition_broadcast(Rb, r_row)
                        if i == 0:
                            nc.vector.tensor_mul(
                                OT[u][0:Dh, q0 + c0:q0 + c1],
                                osb[0:Dh, c0:c1], Rb)
                        else:
                            tmp = rnp.tile([Dh, W], din, tag=f"tmp{W}",
                                           name="tmp")
                            nc.vector.tensor_mul(tmp, osb[0:Dh, c0:c1], Rb)
                            nc.sync.dma_start(
                                out=OT[u][Dh:128, q0 + c0:q0 + c1], in_=tmp)
                    return run

                return [(0.0, norm(0)), (0.0, norm(1))]

            def proj_items(u, qs, tail=False):
                t0 = u * TPU

                def proj(tt, osp):
                    def run():
                        pp = mps.tile([128, QS], f32, tag="m", name="pp")
                        nc.tensor.matmul(
                            pp,
                            OT[u][:, tt * 128:(tt + 1) * 128],
                            wp_sb[:, osp * QS:(osp + 1) * QS],
                            skip_group_check=True,
                        )
                        po = pop.tile([128, QS], f32, name="po")
                        if tail and (tt + osp) % 2:
                            # tail: ACT is idle — split PSUM evictions
                            # between the Scalar and Vector engines
                            nc.scalar.copy(po, pp)
                        else:
                            nc.vector.tensor_copy(po, pp)
                        nc.sync.dma_start(
                            out=out[t0 + tt * 128: t0 + (tt + 1) * 128,
                                    osp * QS:(osp + 1) * QS],
                            in_=po,
                        )
                    return run

                items = []
                for tl in range(QS // 128):
                    tt = qs * (QS // 128) + tl
                    for osp in range(C // QS):
                        items.append((1.0, proj(tt, osp)))
                return items

            # ---- flat software-pipelined emission ----
            # One global stream of 2-kc super-slots across ALL (unit,
            # q-span) boundaries.  av_pending always lags one super-slot
            # behind its exp, INCLUDING across q-span/unit boundaries, so
            # the exp pipeline never drains at a boundary: the finished
            # span's last AVs + oh eviction + normalize + proj all ride
            # inside the next span's slots.
            pending_qkv = [(u, tt) for u in range(1, B) for tt in range(NTT)]
            pending_qkv.reverse()

            fin_box = [None]     # (u, qs, oh) span whose last AVs pend
            tail_box = [None]

            def queue_span_epilogue(pu, pqs, osbs):
                if (pu, pqs) == (B - 1, NQS - 1):
                    tail_box[0] = (pu, pqs, osbs)
                    return
                # normalize is emitted DIRECTLY (all off-PE work) so the
                # rnp/OT WAR windows stay short; only proj rides the
                # budget
                for _, fn in normalize_items(pu, pqs, osbs):
                    fn()
                soft_items.extend(proj_items(pu, pqs))

            def emit_pending_avs():
                for av in av_pending[0]:
                    av()
                av_pending[0] = []
                if fin_box[0] is not None:
                    pu, pqs, poh = fin_box[0]
                    fin_box[0] = None
                    queue_span_epilogue(pu, pqs, evict_oh(poh))

            # pops: unit u's qkv t-tiles drain as filler during earlier
            # spans, finishing well before span (u, 0)
            SPS = NKC // PAIRK          # super-slots per span (8)
            pop_sched = {}
            for uu in range(1, B):
                for k in range(NTT):
                    pop_sched[NQS * SPS * (uu - 1) + POP_START
                              + k * POP_EVERY] = (uu, k)

            si = 0               # global super-slot index
            for u in range(B):
                for qs in range(NQS):
                    if qs == 0 and u > 0:
                        # hard guarantee: unit u's qkv fully emitted
                        # before its attention (emission order is
                        # semantic order for the in-order PE queue)
                        while pending_qkv and pending_qkv[-1][0] == u:
                            nu, ntt = pending_qkv.pop()
                            hard_items.extend(
                                qkv_items(nu, ntt, early_load=True))
                        pump_all_hard()
                    elif qs > 0 and u == 0:
                        # span (0, qs) scores read Q of t-tile qs:
                        # emit its deferred Q matmuls now
                        pump_med_n(NCC)
                    oh = None
                    for j in range(0, NKC, PAIRK):
                        pair = list(range(j, j + PAIRK))
                        if u == 0 and qs == 0 and pair[0] % KPT == 0:
                            # unit-0 startup: at each t-tile's first
                            # k-pair, finish that t-tile's qkv emission
                            # (ordering), then prepush the NEXT t-tile
                            # so the budget drains it inside this
                            # t-tile's slots (its Q deferred to medium)
                            tt = pair[0] // KPT
                            if tt == 0:
                                hard_items.extend(qkv_items(0, 0))
                            pump_all_hard_only()
                            load_wp()
                            if tt + 1 < NTT:
                                h_it, q_it = qkv_items(
                                    0, tt + 1, early_load=True,
                                    split_q=True)
                                hard_items.extend(h_it)
                                med_items.extend(q_it)
                        if si in pop_sched and pending_qkv:
                            nu, ntt = pop_sched[si]
                            if (nu, ntt) in pending_qkv:
                                pending_qkv.remove((nu, ntt))
                                hard_items.extend(
                                    qkv_items(nu, ntt, early_load=True))
                        pts = [emit_scores_exp(u, qs, kc) for kc in pair]
                        pump_budget(FILL_BUDGET if u > 0 or qs > 0
                                    else FILL0_BUDGET)
                        emit_pending_avs()
                        if oh is None:
                            oh = [avps.tile([Dh + 1, QS], f32,
                                            tag=f"av{i}", name=f"oh{i}")
                                  for i in range(2)]
                        av_pending[0] = [make_av(u, oh, kc, pt)
                                         for kc, pt in zip(pair, pts)]
                        if j + PAIRK >= NKC:
                            fin_box[0] = (u, qs, oh)
                        si += 1

            # flush: last span's AVs + eviction, then normalize + proj.
            # The norm chain idles the PE (HAM re-throttles to 1.2 GHz),
            # so keep the clock warm with dummy matmuls while DVE/GpSimd
            # normalize; tail proj evictions split across Scalar+Vector.
            emit_pending_avs()
            pu, pqs, posbs = tail_box[0]
            pitems = proj_items(pu, pqs, tail=True)
            for ci, (c0, c1) in enumerate(((0, QS // 2), (QS // 2, QS))):
                soft_items.extend(normalize_items(pu, pqs, posbs, c0, c1))
                soft_items.extend((0.0, warm_mm) for _ in range(12))
                soft_items.extend(pitems[ci * 4:(ci + 1) * 4])
            pump_all()

    nc.compile()
    return nc


def _shard_inputs(x, w_qkv, w_proj):
    dt = _np_in_dtype()
    xT = np.ascontiguousarray(x.reshape(NT, C).T).astype(dt)
    in_maps = []
    for c in range(NCORES):
        h0, h1 = HPC * c, HPC * c + 1
        rows = []
        for grp in range(3):          # q, k, v
            for h in (h0, h1):
                rows.append(w_qkv[grp * C + h * Dh: grp * C + (h + 1) * Dh])
        wqkvT_c = np.ascontiguousarray(np.concatenate(rows, 0).T).astype(dt)
        wpT_c = np.ascontiguousarray(
            w_proj[:, 2 * Dh * c: 2 * Dh * (c + 1)].T).astype(dt)
        in_maps.append({"xT": xT, "wqkvT": wqkvT_c, "wpT": wpT_c})
    return in_maps


def kernel(x, w_qkv, w_proj, b_proj, _trace=False, _tmpdir=None):
    from concourse import bass_utils

    if "nc" not in _CACHE:
        _CACHE["nc"] = _build_program()
    nc = _CACHE["nc"]

    in_maps = _shard_inputs(
        np.asarray(x, np.float32),
        np.asarray(w_qkv, np.float32),
        np.asarray(w_proj, np.float32),
    )
    res = bass_utils.run_bass_kernel_spmd(
        nc, in_maps, core_ids=list(range(NCORES)),
        trace=_trace, tmpdir=_tmpdir,
    )
    total = res.results[0]["out"].astype(np.float32)
    for c in range(1, NCORES):
        total += res.results[c]["out"]
    total += np.asarray(b_proj, np.float32)[None, :]
    out = total.reshape(B, N, C)
    if _trace:
        return out, res
    return out



# revision 23
# speedup vs baseline: 1.0640x; 1.0640x over previous
"""Multi-head attention (B=4, N=2048, C=1024, H=16, Dh=64) on 8 TRN2 NeuronCores.

Sharding: tensor-parallel over heads — core c owns heads (2c, 2c+1) for all
batches.  Each core computes its 2 heads' QKV projection, attention, and the
partial output projection (contraction over its 128 head-dims of w_proj);
the host sums the 8 partial projections and adds the bias.

Per-core pipeline (unit = one batch of 2048 tokens):
  - host passes xT = x^T [1024, 8192] so channels land on SBUF partitions
  - QT/KT/VT computed as [128(d, 2 heads stacked), t] tiles
  - scores computed TRANSPOSED: ST[k, q] = KT_h.T @ QT_h (contraction d=64,
    two heads row-packed into the PE array: h0 rows 0-63, h1 rows 64-127,
    running concurrently via tile_position row groups)
  - softmax without max-subtraction (scores verified: |s|*scale < 10):
    ACT exp reads the score PSUM pair [128, 1024] directly, writes PT
  - AV: O^T[d, q] with lhsT = [V_h | ones] (M=65): PSUM row 64 accumulates
    the softmax denominator for free; the accumulator is evicted to SBUF
    immediately (frees the PSUM bank) and normalized off the critical path
  - proj: out[t, o] = OT_tile.T @ wpT, evicted right after each q-span

Scheduling: the kernel is a software pipeline clocked by the Scalar
engine's exp (~1.06us per k-chunk).  Attention is emitted in 2-k-chunk
super-slots: both score pairs back-to-back (the second pair's LDWEIGHTS
hides under the first — K=64 row-group reuse), then a cost-budgeted
amount of filler (next unit's QKV / V-transposes / previous q-span's
proj, at SINGLE-matmul granularity), then the AV pairs of the PREVIOUS
super-slot (staggered so the in-order PE queue never waits on ACT).

MHA_DTYPE env: "bf16" (default) or "f32r" or "f32" — matmul input dtype.
PSUM accumulation and softmax statistics are always fp32.
"""

import os
import numpy as np

B, N, C = 4, 2048, 1024
H, Dh = 16, 64
NT = B * N            # 8192 tokens
NCORES = 8
HPC = H // NCORES     # 2 heads per core
SCALE = Dh ** -0.5

TPU = N               # tokens per unit (one batch)
QS = 512              # q-span
KC = 128              # k-chunk
MHA_DTYPE = os.environ.get("MHA_DTYPE", "bf16")
FILL_BUDGET = float(os.environ.get("MHA_FILL", "4.4"))
FILL0_BUDGET = float(os.environ.get("MHA_FILL0", "10.0"))  # unit-0 span
PAIRK = int(os.environ.get("MHA_PAIRK", "2"))      # k-chunks per super-slot
WSPLIT = os.environ.get("MHA_WSPLIT", "1") == "1"  # per-cc weight tiles
WARMUP_MMS = int(os.environ.get("MHA_WARMUP", "16"))
POP_START = int(os.environ.get("MHA_POP_START", "8"))
POP_EVERY = int(os.environ.get("MHA_POP_EVERY", "5"))

_CACHE = {}


def _np_in_dtype():
    if MHA_DTYPE == "bf16":
        import ml_dtypes
        return np.dtype(ml_dtypes.bfloat16)
    return np.dtype(np.float32)


def _build_program():
    import concourse.bacc as bacc
    import concourse.bass as bass
    import concourse.tile as tile
    from concourse import mybir
    from concourse.masks import make_identity

    f32 = mybir.dt.float32
    din = {
        "bf16": mybir.dt.bfloat16,
        "f32r": mybir.dt.float32r,
        "f32": mybir.dt.float32,
    }[MHA_DTYPE]

    nc = bacc.Bacc("TRN2", target_bir_lowering=False, debug=False)

    xT = nc.dram_tensor("xT", [C, NT], din, kind="ExternalInput").ap()
    wqkvT = nc.dram_tensor("wqkvT", [C, 6 * Dh], din, kind="ExternalInput").ap()
    wpT = nc.dram_tensor("wpT", [2 * Dh, C], din, kind="ExternalInput").ap()
    out = nc.dram_tensor("out", [NT, C], f32, kind="ExternalOutput").ap()

    NCC = C // 128        # 8 c-chunks
    NTT = TPU // QS       # 4 t-tiles per unit
    NKC = TPU // KC       # 16 k-chunks per unit
    NQS = TPU // QS       # 4 q-spans per unit
    KPT = QS // KC        # 4 k-chunks per t-tile
    VW = 2 * (Dh + 1)     # 130: V_sb row layout [V_h0 | 1 | V_h1 | 1]

    with tile.TileContext(nc) as tc:
        with (
            tc.tile_pool(name="const", bufs=1) as const,
            tc.tile_pool(name="xp", bufs=48) as xp,
            tc.tile_pool(name="qt", bufs=2) as qtp,
            tc.tile_pool(name="kt", bufs=2) as ktp,
            tc.tile_pool(name="vt", bufs=2) as vtp,
            tc.tile_pool(name="vsb", bufs=2) as vsbp,
            tc.tile_pool(name="pt", bufs=4) as ptp,
            tc.tile_pool(name="ot", bufs=2) as otp,
            tc.tile_pool(name="rn", bufs=2) as rnp,
            tc.tile_pool(name="po", bufs=4) as pop,
            tc.tile_pool(name="mps", bufs=2, space="PSUM") as mps,
            tc.tile_pool(name="stps", bufs=2, space="PSUM") as stps,
            tc.tile_pool(name="avps", bufs=1, space="PSUM") as avps,
        ):
            ident = const.tile([128, 128], din)
            make_identity(nc, ident)

            # ramp: pull the ACT exp-table load (~2.7us) off the critical
            # path, and keep the PE busy during the initial x/w DMAs so the
            # HAM clock-gate is released before the first real matmul
            def warm_mm():
                # real matmul (transpose-mode does NOT engage the HAM
                # clock-gate): keeps/brings the PE at 2.4 GHz
                wps = mps.tile([128, 128], f32, tag="m", name="wps")
                nc.tensor.matmul(wps, ident, ident, skip_group_check=True)

            if WARMUP_MMS > 0:
                warm_out = const.tile([128, 8], f32)
                nc.scalar.activation(
                    warm_out, ident[:, 0:8],
                    mybir.ActivationFunctionType.Exp)
                for _ in range(WARMUP_MMS):
                    warm_mm()

            # per-cc weight tiles: first QKV matmul only waits on its own chunk
            wq_sbs = []
            if WSPLIT:
                for cc in range(NCC):
                    w_t = const.tile([128, 6 * Dh], din, tag=f"w_t{cc}",
                                     name=f"w_t{cc}")
                    nc.gpsimd.dma_start(
                        out=w_t,
                        in_=wqkvT[cc * 128:(cc + 1) * 128, :],
                    )
                    wq_sbs.append(w_t)
            else:
                wq_sb = const.tile([128, NCC * 6 * Dh], din)
                for cc in range(NCC):
                    nc.gpsimd.dma_start(
                        out=wq_sb[:, cc * 6 * Dh:(cc + 1) * 6 * Dh],
                        in_=wqkvT[cc * 128:(cc + 1) * 128, :],
                    )
                    wq_sbs.append(wq_sb[:, cc * 6 * Dh:(cc + 1) * 6 * Dh])
            # wp DMA is issued AFTER unit-0 t-tile-0's x DMAs (below):
            # same gpsimd queue, and wp isn't needed until the first proj
            wp_sb = const.tile([128, C], din)
            wp_loaded = [False]

            def load_wp():
                if not wp_loaded[0]:
                    wp_loaded[0] = True
                    nc.gpsimd.dma_start(out=wp_sb, in_=wpT)

            # per-unit persistent tiles, allocated lazily
            QT, KT, VT, VSB, OT = {}, {}, {}, {}, {}

            def alloc_unit(u):
                QT[u] = qtp.tile([128, TPU], din, tag="QT", name=f"QT{u}")
                KT[u] = ktp.tile([128, TPU], din, tag="KT", name=f"KT{u}")
                VT[u] = vtp.tile([128, TPU], din, tag="VT", name=f"VT{u}")
                VSB[u] = vsbp.tile([128, NKC * VW], din, tag="VSB", name=f"VSB{u}")

            def qkv_items(u, tt, early_load=False, split_q=False):
                """QKV projection + V transposes for t-tile tt of unit u, as
                (cost, closure) items at single-matmul granularity.  With
                early_load the x DMAs are issued immediately (slots before
                the matmuls drain) so their latency is hidden instead of
                stalling the first matmul of the group.

                Groups are emitted K, V, transposes, Q: attention on q-span
                qs only reads the Q of t-tile qs, but the K/V of ALL
                t-tiles, so K/V are the critical-path part.  With split_q
                the Q matmuls are returned separately (lower priority:
                unit-0 t-tile tt's Q is not needed until span (0, tt))."""
                items = []
                q_items = []
                state = {}

                def load_x():
                    if tt == 0:
                        alloc_unit(u)
                    t0 = u * TPU
                    cells = []
                    for cc in range(NCC):
                        xt = xp.tile([128, QS], din, tag="xs", name="xt")
                        # sync queue (NOT gpsimd: interleaving DMA
                        # triggers with partition_broadcast forces a
                        # ~5us gpsimd library swap per alternation)
                        nc.sync.dma_start(
                            out=xt,
                            in_=xT[cc * 128:(cc + 1) * 128,
                                   t0 + tt * QS:t0 + (tt + 1) * QS],
                        )
                        cells.append(xt)
                    state["xs"] = cells

                if early_load:
                    load_x()
                else:
                    items.append((0.0, load_x))

                def qkv_mm(grp, cc):
                    def run():
                        if cc == 0:
                            state[grp] = mps.tile([128, QS], f32, tag="m",
                                                  name="ps")
                        ps = state[grp]
                        w_sl = wq_sbs[cc][:, grp * 128:(grp + 1) * 128]
                        nc.tensor.matmul(
                            ps, w_sl, state["xs"][cc],
                            start=(cc == 0), stop=(cc == NCC - 1),
                            skip_group_check=True,
                        )
                        if cc == NCC - 1:
                            tgt = (QT, KT, VT)[grp][u]
                            nc.vector.tensor_copy(
                                tgt[:, tt * QS:(tt + 1) * QS], ps)
                    return run

                for cc in range(NCC):
                    items.append((1.0, qkv_mm(1, cc)))     # K
                if u == 0 and tt == 0:
                    # startup critical path: the first scores need K+Q
                    for cc in range(NCC):
                        items.append((1.0, qkv_mm(0, cc)))  # Q
                for cc in range(NCC):
                    items.append((1.0, qkv_mm(2, cc)))     # V
                if not (u == 0 and tt == 0):
                    for cc in range(NCC):
                        (q_items if split_q else items).append(
                            (1.0, qkv_mm(0, cc)))          # Q

                def transpose_item(j):
                    def run():
                        kc = tt * KPT + j
                        tp = mps.tile([128, 128], din, tag="m", name="tp")
                        nc.tensor.transpose(
                            tp, VT[u][:, kc * 128:(kc + 1) * 128], ident)
                        base = kc * VW
                        nc.vector.tensor_copy(
                            VSB[u][:, base: base + Dh], tp[:, 0:Dh])
                        nc.vector.memset(
                            VSB[u][:, base + Dh: base + Dh + 1], 1.0)
                        nc.vector.tensor_copy(
                            VSB[u][:, base + Dh + 1: base + 2 * Dh + 1],
                            tp[:, Dh: 2 * Dh])
                        nc.vector.memset(
                            VSB[u][:, base + 2 * Dh + 1: base + VW], 1.0)
                    return run

                # transposes right after V (before Q): AVs need VSB one
                # slot after their scores
                if u == 0 and tt == 0:
                    for j in range(KPT):
                        items.append((0.8, transpose_item(j)))
                else:
                    base = (0 if early_load else 1) + 2 * NCC
                    for j in range(KPT):
                        items.insert(base + j, (0.8, transpose_item(j)))
                if split_q:
                    return items, q_items
                return items

            hard_items = []   # qkv work: must drain before its unit's attn
            med_items = []    # unit-0 deferred Q matmuls
            soft_items = []   # normalize/proj: anytime
            carry = [0.0]     # fractional pump budget carry

            def pump_budget(budget):
                """Pop items worth ~budget matmul-equivalents of PE time."""
                c = carry[0] + budget
                while c > 0 and (hard_items or med_items or soft_items):
                    q = (hard_items if hard_items
                         else med_items if med_items else soft_items)
                    cost, fn = q.pop(0)
                    fn()
                    c -= cost if cost > 0 else 0.0
                    if cost == 0.0:
                        continue
                carry[0] = (min(c, 0.0)
                            if (hard_items or med_items or soft_items)
                            else 0.0)

            def pump_all_hard_only():
                while hard_items:
                    _, fn = hard_items.pop(0)
                    fn()

            def pump_all_hard():
                pump_all_hard_only()
                while med_items:
                    _, fn = med_items.pop(0)
                    fn()

            def pump_med_n(n):
                while med_items and n > 0:
                    _, fn = med_items.pop(0)
                    fn()
                    n -= 1

            def pump_all():
                pump_all_hard()
                while soft_items:
                    _, fn = soft_items.pop(0)
                    fn()

            # ---- attention: 2-k-chunk super-slots, AVs staggered one
            # super-slot behind their exp so PE never stalls on ACT; score
            # pairs of adjacent k-chunks run back-to-back so the second
            # pair's LDWEIGHTS hides under the first (K=64 row-group reuse)
            av_pending = [[]]

            def emit_scores_exp(u, qs, kc):
                q0 = qs * QS
                sp = stps.tile([128, 2 * QS], f32, name="sp")
                nc.tensor.matmul(
                    sp[:, 0:QS],
                    KT[u][0:Dh, kc * 128:(kc + 1) * 128],
                    QT[u][0:Dh, q0:q0 + QS],
                    skip_group_check=True,
                )
                nc.tensor.matmul(
                    sp[:, QS:2 * QS],
                    KT[u][Dh:128, kc * 128:(kc + 1) * 128],
                    QT[u][Dh:128, q0:q0 + QS],
                    skip_group_check=True,
                )
                pt = ptp.tile([128, 2 * QS], din, name="pt")
                nc.scalar.activation(
                    pt, sp, mybir.ActivationFunctionType.Exp,
                    scale=SCALE,
                )
                return pt

            def make_av(u, oh, kc, pt):
                def run():
                    for i in range(2):
                        vbase = kc * VW + i * (Dh + 1)
                        nc.tensor.matmul(
                            oh[i],
                            VSB[u][:, vbase: vbase + Dh + 1],
                            pt[:, i * QS:(i + 1) * QS],
                            start=(kc == 0), stop=(kc == NKC - 1),
                            skip_group_check=True,
                        )
                return run

            def evict_oh(oh):
                """Evict AV accumulators to SBUF (frees PSUM)."""
                osbs = []
                for i in range(2):
                    osb = rnp.tile([Dh + 1, QS], f32, tag=f"osb{i}",
                                   name=f"osb{i}")
                    nc.vector.tensor_copy(osb, oh[i])
                    osbs.append(osb)
                return osbs

            def normalize_items(u, qs, osbs, c0=0, c1=QS, warm=False):
                if qs == 0 and c0 == 0:
                    OT[u] = otp.tile([128, TPU], din, tag="OT",
                                     name=f"OT{u}")
                q0 = qs * QS
                W = c1 - c0

                def norm(i):
                    def run():
                        osb = osbs[i]
                        d_row = rnp.tile([1, W], f32, tag=f"d{W}",
                                         name="d_row")
                        nc.vector.tensor_copy(d_row, osb[Dh:Dh + 1, c0:c1])
                        r_row = rnp.tile([1, W], f32, tag=f"r{W}",
                                         name="r_row")
                        nc.vector.reciprocal_approx_fast(r_row, d_row)
                        Rb = rnp.tile([Dh, W], f32, tag=f"R{W}", name="Rb")
                        nc.gpsimd.partition_broadcast(Rb, r_row)
                        if i == 0:
                            nc.vector.tensor_mul(
                                OT[u][0:Dh, q0 + c0:q0 + c1],
                                osb[0:Dh, c0:c1], Rb)
                        else:
                            tmp = rnp.tile([Dh, W], din, tag=f"tmp{W}",
                                           name="tmp")
                            nc.vector.tensor_mul(tmp, osb[0:Dh, c0:c1], Rb)
                            nc.sync.dma_start(
                                out=OT[u][Dh:128, q0 + c0:q0 + c1], in_=tmp)
                    return run

                return [(0.0, norm(0)), (0.0, norm(1))]

            def proj_items(u, qs, tail=False):
                t0 = u * TPU

                def proj(tt, osp):
                    def run():
                        pp = mps.tile([128, QS], f32, tag="m", name="pp")
                        nc.tensor.matmul(
                            pp,
                            OT[u][:, tt * 128:(tt + 1) * 128],
                            wp_sb[:, osp * QS:(osp + 1) * QS],
                            skip_group_check=True,
                        )
                        po = pop.tile([128, QS], f32, name="po")
                        if tail and (tt + osp) % 2:
                            # tail: ACT is idle — split PSUM evictions
                            # between the Scalar and Vector engines
                            nc.scalar.copy(po, pp)
                        else:
                            nc.vector.tensor_copy(po, pp)
                        nc.sync.dma_start(
                            out=out[t0 + tt * 128: t0 + (tt + 1) * 128,
                                    osp * QS:(osp + 1) * QS],
                            in_=po,
                        )
                    return run

                items = []
                for tl in range(QS // 128):
                    tt = qs * (QS // 128) + tl
                    for osp in range(C // QS):
                        items.append((1.0, proj(tt, osp)))
                return items

            # ---- flat software-pipelined emission ----
            # One global stream of 2-kc super-slots across ALL (unit,
            # q-span) boundaries.  av_pending always lags one super-slot
            # behind its exp, INCLUDING across q-span/unit boundaries, so
            # the exp pipeline never drains at a boundary: the finished
            # span's last AVs + oh eviction + normalize + proj all ride
            # inside the next span's slots.
            pending_qkv = [(u, tt) for u in range(1, B) for tt in range(NTT)]
            pending_qkv.reverse()

            fin_box = [None]     # (u, qs, oh) span whose last AVs pend
            tail_box = [None]

            def queue_span_epilogue(pu, pqs, osbs):
                if (pu, pqs) == (B - 1, NQS - 1):
                    tail_box[0] = (pu, pqs, osbs)
                    return
                # normalize is emitted DIRECTLY (all off-PE work) so the
                # rnp/OT WAR windows stay short; only proj rides the
                # budget
                for _, fn in normalize_items(pu, pqs, osbs):
                    fn()
                soft_items.extend(proj_items(pu, pqs))

            def emit_pending_avs():
                for av in av_pending[0]:
                    av()
                av_pending[0] = []
                if fin_box[0] is not None:
                    pu, pqs, poh = fin_box[0]
                    fin_box[0] = None
                    queue_span_epilogue(pu, pqs, evict_oh(poh))

            # pops: unit u's qkv t-tiles drain as filler during earlier
            # spans, finishing well before span (u, 0)
            SPS = NKC // PAIRK          # super-slots per span (8)
            pop_sched = {}
            for uu in range(1, B):
                for k in range(NTT):
                    pop_sched[NQS * SPS * (uu - 1) + POP_START
                              + k * POP_EVERY] = (uu, k)

            si = 0               # global super-slot index
            for u in range(B):
                for qs in range(NQS):
                    if qs == 0 and u > 0:
                        # hard guarantee: unit u's qkv fully emitted
                        # before its attention (emission order is
                        # semantic order for the in-order PE queue)
                        while pending_qkv and pending_qkv[-1][0] == u:
                            nu, ntt = pending_qkv.pop()
                            hard_items.extend(
                                qkv_items(nu, ntt, early_load=True))
                        pump_all_hard()
                    elif qs > 0 and u == 0:
                        # span (0, qs) scores read Q of t-tile qs:
                        # emit its deferred Q matmuls now
                        pump_med_n(NCC)
                    oh = None
                    for j in range(0, NKC, PAIRK):
                        pair = list(range(j, j + PAIRK))
                        if u == 0 and qs == 0 and pair[0] % KPT == 0:
                            # unit-0 startup: at each t-tile's first
                            # k-pair, finish that t-tile's qkv emission
                            # (ordering), then prepush the NEXT t-tile
                            # so the budget drains it inside this
                            # t-tile's slots (its Q deferred to medium)
                            tt = pair[0] // KPT
                            if tt == 0:
                                hard_items.extend(qkv_items(0, 0))
                            pump_all_hard_only()
                            load_wp()
                            if tt + 1 < NTT:
                                h_it, q_it = qkv_items(
                                    0, tt + 1, early_load=True,
                                    split_q=True)
                                hard_items.extend(h_it)
                                med_items.extend(q_it)
                        if si in pop_sched and pending_qkv:
                            nu, ntt = pop_sched[si]
                            if (nu, ntt) in pending_qkv:
                                pending_qkv.remove((nu, ntt))
                                hard_items.extend(
                                    qkv_items(nu, ntt, early_load=True))
                        pts = [emit_scores_exp(u, qs, kc) for kc in pair]
                        pump_budget(FILL_BUDGET if u > 0 or qs > 0
                                    else FILL0_BUDGET)
                        emit_pending_avs()
                        if oh is None:
                            oh = [avps.tile([Dh + 1, QS], f32,
                                            tag=f"av{i}", name=f"oh{i}")
                                  for i in range(2)]
                        av_pending[0] = [make_av(u, oh, kc, pt)
                                         for kc, pt in zip(pair, pts)]
                        if j + PAIRK >= NKC:
                            fin_box[0] = (u, qs, oh)
                        si += 1

            # flush: last span's AVs + eviction, then normalize + proj.
            # The norm chain idles the PE (HAM re-throttles to 1.2 GHz),
            # so keep the clock warm with dummy matmuls while DVE/GpSimd
            # normalize; tail proj evictions split across Scalar+Vector.
            emit_pending_avs()
            pu, pqs, posbs = tail_box[0]
            pitems = proj_items(pu, pqs, tail=True)
            for ci, (c0, c1) in enumerate(((0, QS // 2), (QS // 2, QS))):
                soft_items.extend(normalize_items(pu, pqs, posbs, c0, c1))
                soft_items.extend((0.0, warm_mm) for _ in range(12))
                soft_items.extend(pitems[ci * 4:(ci + 1) * 4])
            pump_all()

    nc.compile()
    return nc


def _shard_inputs(x, w_qkv, w_proj):
    dt = _np_in_dtype()
    xT = np.ascontiguousarray(x.reshape(NT, C).T).astype(dt)
    in_maps = []
    for c in range(NCORES):
        h0, h1 = HPC * c, HPC * c + 1
        rows = []
        for grp in range(3):          # q, k, v
            for h in (h0, h1):
                rows.append(w_qkv[grp * C + h * Dh: grp * C + (h + 1) * Dh])
        wqkvT_c = np.ascontiguousarray(np.concatenate(rows, 0).T).astype(dt)
        wpT_c = np.ascontiguousarray(
            w_proj[:, 2 * Dh * c: 2 * Dh * (c + 1)].T).astype(dt)
        in_maps.append({"xT": xT, "wqkvT": wqkvT_c, "wpT": wpT_c})
    return in_maps


def kernel(x, w_qkv, w_proj, b_proj, _trace=False, _tmpdir=None):
    from concourse import bass_utils

    if "nc" not in _CACHE:
        _CACHE["nc"] = _build_program()
    nc = _CACHE["nc"]

    in_maps = _shard_inputs(
        np.asarray(x, np.float32),
        np.asarray(w_qkv, np.float32),
        np.asarray(w_proj, np.float32),
    )
    res = bass_utils.run_bass_kernel_spmd(
        nc, in_maps, core_ids=list(range(NCORES)),
        trace=_trace, tmpdir=_tmpdir,
    )
    total = res.results[0]["out"].astype(np.float32)
    for c in range(1, NCORES):
        total += res.results[c]["out"]
    total += np.asarray(b_proj, np.float32)[None, :]
    out = total.reshape(B, N, C)
    if _trace:
        return out, res
    return out

